# revision 7
# baseline (speedup 1.0000x reference)
"""Trainium2 Bass kernel for the LIF-network step (nn_NetworkClass_31018253812098).

Computation (reference, all fp32, N = NN = N_IN = 2048):
    z_out_new = BETA * z_out + z
    v_new     = ALPHA * v + x @ w - V_TH * z + z_out_new @ wrec
    mask      = (v_new[0, :] - V_TH) > 0          # length-2048, from batch row 0
    z_new[i, j] = mask[i]                         # row-broadcast (N == NN)

Strategy: 4x2 grid -- 4 batch shards (512 cols) x 2 feature halves (1024
rows) -- in the TRANSPOSED domain on-chip: per-core tensors are stored
[feature, batch] so the contraction dim of both matmuls lands on SBUF
partitions natively (w / wrec stay natural as the stationary operands,
column-halved per core).  The whole datapath is bfloat16 in HBM: the PE
runs bf16 at the same 1 column/cycle as fp32r, so quantizing everything
to bf16 halves DMA traffic at ~2.6e-3 relative error, far inside the
2e-2 gate.  PSUM accumulation and the threshold compare stay fp32.

The batch shard is exactly 512 columns, so each matmul's moving dim fills
one 2 KiB PSUM bank exactly: the two output quarters use disjoint banks
and the PE never waits on an epilogue.  Matmul phases run MM1q0, MM1q1,
MM2q0, MM2q1: the second phase reuses the resident x, so the z/z_out
streams for MM2's rhs (zon) have a whole extra phase to arrive -- this
makes the DMA-prefix deadline chain start-limited rather than
mid-stream-limited.  Weights stream on the scalar engine's HWDGE ring
while activations stream on sync's, so the two issue sequencers run in
parallel (each dma_start costs ~0.6 us of issue time) and the first
weight piece lands concurrently with the first x tiles.  Load
granularity follows the deadline slack: fine pieces where the PE is
supply-limited (start of MM1q0), 1 MiB pieces where there is multi-us
slack (wrec[1], other-half z streams) -- fewer dma_starts also shrink
the post-kernel notification flush, which scales with event count.
Stores go on the gpsimd SWDGE.  The final MM2q1 chunk runs n-major so
each PSUM bank completes early; the per-tile epilogue does the fp32
threshold compare first, then one fused op (v_out = ALPHA*pre + psum,
with pre = v - (V_TH/ALPHA)*z built mid-stream), so the mask store and
split v_out stores overlap the last matmuls.  Batch row 0 is column 0 of
the ms==0 shard, so those cores compute the real mask; z_new is rank-1
and only the [P, HT] mask leaves the device (host broadcasts it).  SPMD
uniformity across the feature halves is achieved purely in DATA: the
host permutes the tile order of z/z_out (own half first) and permutes
wrec's row blocks to match, so one program serves both halves.
"""

import sys

sys.path.insert(0, "/opt/trn_rl_repo")

import ml_dtypes
import numpy as np

import concourse.mybir as mybir
import concourse.tile as tile
from concourse import bacc, bass_utils

N = 2048
P = 128
NT = N // P          # 16 feature/contraction tiles
NCORES = 8
R, C = 4, 2          # batch shards x feature halves
MS = N // R          # 512-column batch shard == one PSUM bank of fp32
NH = N // C          # 1024-row feature half
HT = NH // P         # 8 n-tiles per half
KW = 8               # k-tiles per packed weight chunk (1 MiB chunks)
ALPHA = 1.0 - 0.05 / 10.0   # 0.995
BETA = 1.0 - 0.05 / 2.0     # 0.975
V_TH = 2.0

F32 = mybir.dt.float32
BF16 = mybir.dt.bfloat16
NP_BF16 = ml_dtypes.bfloat16


def _build_program():
    # bacc (not raw Bass): its compile pass splits multi-semaphore sync
    # waits that walrus's per-instruction wait limit rejects.
    nc = bacc.Bacc("TRN2", target_bir_lowering=False, debug=False, num_devices=NCORES)

    xt = nc.dram_tensor("xt", [P, NT, MS], BF16, kind="ExternalInput").ap()
    vt = nc.dram_tensor("vt", [P, HT, MS], BF16, kind="ExternalInput").ap()
    zt = nc.dram_tensor("zt", [P, NT, MS], BF16, kind="ExternalInput").ap()
    zot = nc.dram_tensor("zot", [P, NT, MS], BF16, kind="ExternalInput").ap()
    # chunk-major: [quarter, chunk, p, a, n] in exact DMA consumption order
    wh = nc.dram_tensor("wh", [2, NT // KW, P, KW, MS], BF16, kind="ExternalInput").ap()
    wrech = nc.dram_tensor(
        "wrech", [2, NT // KW, P, KW, MS], BF16, kind="ExternalInput"
    ).ap()

    vout = nc.dram_tensor("vout", [P, HT, MS], BF16, kind="ExternalOutput").ap()
    zoout = nc.dram_tensor("zoout", [P, HT, MS], BF16, kind="ExternalOutput").ap()
    maskout = nc.dram_tensor("maskout", [P, HT], F32, kind="ExternalOutput").ap()

    add = mybir.AluOpType.add
    mult = mybir.AluOpType.mult
    is_gt = mybir.AluOpType.is_gt

    with tile.TileContext(nc) as tc:
        with (
            tc.tile_pool(name="resident", bufs=1) as res,
            tc.tile_pool(name="zstream", bufs=2) as zs,
            tc.tile_pool(name="w2", bufs=2) as w2p,
            tc.tile_pool(name="w4", bufs=6) as w4p,
            tc.tile_pool(name="w8", bufs=2) as w8p,
            tc.tile_pool(name="psum", bufs=8, space="PSUM") as psum_pool,
            tc.tile_pool(name="epi", bufs=2) as epi,
        ):
            xt_s = res.tile([P, NT, MS], BF16, tag="xt_s")
            zt_s = res.tile([P, HT, MS], BF16, tag="zt_s")      # own half only
            vt_s = res.tile([P, HT, MS], BF16, tag="vt_s")
            zon = res.tile([P, NT, MS], BF16, tag="zon")        # matmul-2 rhs
            zot01 = res.tile([P, HT, MS], BF16, tag="zot01")    # own-half z_out
            pre = res.tile([P, HT, MS], F32, tag="pre")         # v - (V_TH/ALPHA)*z
            mask_s = res.tile([P, HT], F32, tag="mask_s")

            ps0 = [psum_pool.tile([P, MS], F32, tag="ps", name=f"ps0_{i}") for i in range(4)]
            ps1 = [psum_pool.tile([P, MS], F32, tag="ps", name=f"ps1_{i}") for i in range(4)]

            POOLS = {2: w2p, 4: w4p, 8: w8p}

            def wload(src, q, ci, a0, a1):
                """One weight piece [P, a1-a0, MS] on the scalar ring."""
                wc = POOLS[a1 - a0].tile([P, a1 - a0, MS], BF16, tag=f"wc{a1 - a0}")
                nc.scalar.dma_start(wc[:], src[q, ci, :, a0:a1, :])
                return wc

            def mm(wc, a, k, ps_n, rhs, start, stop):
                for n in range(4):
                    nc.tensor.matmul(
                        ps_n[n][:],
                        lhsT=wc[:, a, n * P : (n + 1) * P],
                        rhs=rhs[:, k, :],
                        start=start,
                        stop=stop,
                    )

            # --- MM1 q0: fine pieces, supply-limited phase.  Weight pieces
            # on the scalar ring land concurrently with x tiles on sync's.
            W0 = [(0, 0, 2), (0, 2, 4), (0, 4, 8), (1, 0, 4), (1, 4, 8)]
            XT = [(0, 2), (2, 4), (4, 8), (8, 12), (12, 16)]
            k = 0
            for i, (ci, a0, a1) in enumerate(W0):
                wc = wload(wh, 0, ci, a0, a1)
                t0, t1 = XT[i]
                nc.sync.dma_start(xt_s[:, t0:t1, :], xt[:, t0:t1, :])
                for a in range(a1 - a0):
                    mm(wc, a, k, ps0, xt_s, start=(k == 0), stop=False)
                    k += 1

            # --- MM1 q1: xt already resident, only wh[1] streams ---
            W1 = [(0, 0, 4), (0, 4, 8), (1, 0, 4), (1, 4, 8)]
            k = 0
            for ci, a0, a1 in W1:
                wc = wload(wh, 1, ci, a0, a1)
                for a in range(a1 - a0):
                    mm(wc, a, k, ps1, xt_s, start=(k == 0), stop=False)
                    k += 1

            # --- zon inputs (sync ring) + own-half builds ---
            nc.sync.dma_start(zt_s[:], zt[:, 0:HT, :])
            nc.sync.dma_start(zot01[:], zot[:, 0:HT, :])
            for j in range(HT):
                nc.vector.scalar_tensor_tensor(
                    zon[:, j, :], zot01[:, j, :], BETA, zt_s[:, j, :], mult, add
                )

            # --- MM2 q0: other-half z streams (1 MiB, multi-us slack) +
            # wrec[0] in 512 KiB pieces ---
            zt_q = zs.tile([P, HT, MS], BF16, tag="zt_q")
            nc.sync.dma_start(zt_q[:], zt[:, HT:NT, :])
            zot_q = zs.tile([P, HT, MS], BF16, tag="zot_q")
            nc.sync.dma_start(zot_q[:], zot[:, HT:NT, :])
            for j in range(HT):
                nc.vector.scalar_tensor_tensor(
                    zon[:, HT + j, :], zot_q[:, j, :], BETA, zt_q[:, j, :], mult, add
                )
            k = 0
            for ci, a0, a1 in W1:
                wc = wload(wrech, 0, ci, a0, a1)
                for a in range(a1 - a0):
                    mm(wc, a, k, ps0, zon, start=False, stop=(k == NT - 1))
                    k += 1

            nc.sync.dma_start(vt_s[:], vt[:])
            # pre = v - (V_TH/ALPHA)*z; epilogue applies *ALPHA so that
            # ALPHA*pre + psum = ALPHA*v - V_TH*z + matmuls
            for t in range(HT):
                nc.vector.scalar_tensor_tensor(
                    pre[:, t, :], zt_s[:, t, :], -V_TH / ALPHA, vt_s[:, t, :], mult, add
                )

            def epi_tile(t, psn, vob, nv):
                # fp32 threshold compare first, so the mask store need not
                # wait for the full-width add
                u = epi.tile([P, 1], F32, tag="u")
                nc.vector.scalar_tensor_tensor(
                    u[:], pre[:, t, 0:1], ALPHA, psn[:, 0:1], mult, add
                )
                nc.vector.tensor_scalar(mask_s[:, t : t + 1], u[:], V_TH, None, is_gt)
                nc.vector.scalar_tensor_tensor(
                    vob[:, nv, :], pre[:, t, :], ALPHA, psn[:], mult, add
                )

            vob0 = epi.tile([P, 4, MS], BF16, tag="vob", name="vob0")
            for n in range(4):
                epi_tile(n, ps0[n], vob0, n)
            nc.gpsimd.dma_start(zoout[:], zon[:, 0:HT, :])
            nc.gpsimd.dma_start(vout[:, 0:4, :], vob0[:])

            # --- MM2 q1: wrec[1] in 1 MiB pieces (multi-us deadline slack);
            # the final chunk runs n-major so each PSUM bank finishes early
            # and the epilogue/stores overlap the last matmuls ---
            wc = wload(wrech, 1, 0, 0, KW)
            for a in range(KW):
                mm(wc, a, a, ps1, zon, start=False, stop=False)
            wc = wload(wrech, 1, 1, 0, KW)
            vob1 = epi.tile([P, 4, MS], BF16, tag="vob", name="vob1")
            for n in range(4):
                for a in range(KW):
                    nc.tensor.matmul(
                        ps1[n][:],
                        lhsT=wc[:, a, n * P : (n + 1) * P],
                        rhs=zon[:, KW + a, :],
                        start=False,
                        stop=(a == KW - 1),
                    )
                epi_tile(4 + n, ps1[n], vob1, n)
                if n == 1:
                    nc.gpsimd.dma_start(vout[:, 4:6, :], vob1[:, 0:2, :])
            nc.gpsimd.dma_start(maskout[:], mask_s[:])
            nc.gpsimd.dma_start(vout[:, 6:8, :], vob1[:, 2:4, :])

    nc.compile()
    return nc


_PROGRAM_CACHE = {}


def _get_program():
    if "nc" not in _PROGRAM_CACHE:
        _PROGRAM_CACHE["nc"] = _build_program()
    return _PROGRAM_CACHE["nc"]


def _pack(aT, ms, tile_perm=None):
    """[2048, 2048] transposed-domain array -> p-major [128, T, MS] bf16."""
    a = aT[:, ms * MS : (ms + 1) * MS]  # [2048, MS]
    t = a.reshape(-1, P, MS)  # [T, 128, MS]
    if tile_perm is not None:
        t = t[tile_perm]
    return np.ascontiguousarray(t.transpose(1, 0, 2).astype(NP_BF16))


def _pack_w(w_h):
    """[2048, 1024] weight half -> chunk-major [2, 2, 128, KW, MS] bf16."""
    # w_h[ci*1024 + a*128 + p, q*512 + n] -> wp[q, ci, p, a, n]
    t = w_h.reshape(NT // KW, KW, P, 2, MS)
    return np.ascontiguousarray(t.transpose(3, 0, 2, 1, 4).astype(NP_BF16))


def make_in_maps(x, v, z, z_out, w, wrec):
    xT = np.ascontiguousarray(x.T)
    vT = np.ascontiguousarray(v.T)
    zT = np.ascontiguousarray(z.T)
    zoT = np.ascontiguousarray(z_out.T)
    w = np.ascontiguousarray(w, dtype=np.float32)
    wrec = np.ascontiguousarray(wrec, dtype=np.float32)

    wh_packed = [_pack_w(w[:, nh * NH : (nh + 1) * NH]) for nh in range(C)]
    wrech_packed = []
    for nh in range(C):
        perm = np.r_[nh * HT : nh * HT + HT, (1 - nh) * HT : (1 - nh) * HT + HT]
        wr = wrec.reshape(NT, P, N)[perm].reshape(N, N)[:, nh * NH : (nh + 1) * NH]
        wrech_packed.append(_pack_w(wr))

    in_maps = []
    for c in range(NCORES):
        nh, ms = divmod(c, R)
        perm = np.r_[nh * HT : nh * HT + HT, (1 - nh) * HT : (1 - nh) * HT + HT]
        in_maps.append(
            {
                "xt": _pack(xT, ms),
                "vt": _pack(vT, ms)[:, nh * HT : nh * HT + HT],
                "zt": _pack(zT, ms, perm),
                "zot": _pack(zoT, ms, perm),
                "wh": wh_packed[nh],
                "wrech": wrech_packed[nh],
            }
        )
    return in_maps


def gather(results):
    v_new = np.empty((N, N), np.float32)
    z_new = np.empty((N, N), np.float32)
    z_out_new = np.empty((N, N), np.float32)
    for c, r in enumerate(results):
        nh, ms = divmod(c, R)
        rows = slice(nh * NH, (nh + 1) * NH)
        cols = slice(ms * MS, (ms + 1) * MS)
        vo = np.asarray(r["vout"]).astype(np.float32).transpose(1, 0, 2).reshape(NH, MS)
        zo = np.asarray(r["zoout"]).astype(np.float32).transpose(1, 0, 2).reshape(NH, MS)
        v_new[cols, rows] = vo.T  # transposed domain -> natural
        z_out_new[cols, rows] = zo.T
        if ms == 0:
            # batch row 0 is column 0 of this shard: its mask is the real one.
            # mask feature t*128+p lives at maskout[p, t]; z_new is rank-1.
            mvec = np.asarray(r["maskout"]).astype(np.float32).T.reshape(NH)
            z_new[rows, :] = mvec[:, None]
    return v_new, z_new, z_out_new


def kernel(x, v, z, z_out, w, wrec, _trace=False):
    nc = _get_program()
    in_maps = make_in_maps(x, v, z, z_out, w, wrec)
    res = bass_utils.run_bass_kernel_spmd(
        nc, in_maps, core_ids=list(range(NCORES)), trace=_trace
    )
    out = gather(res.results)
    if _trace:
        return out, res
    return out


# revision 8
# speedup vs baseline: 1.0040x; 1.0040x over previous
"""Trainium2 Bass kernel for the LIF-network step (nn_NetworkClass_31018253812098).

Computation (reference, all fp32, N = NN = N_IN = 2048):
    z_out_new = BETA * z_out + z
    v_new     = ALPHA * v + x @ w - V_TH * z + z_out_new @ wrec
    mask      = (v_new[0, :] - V_TH) > 0          # length-2048, from batch row 0
    z_new[i, j] = mask[i]                         # row-broadcast (N == NN)

Strategy: 4x2 grid -- 4 batch shards (512 cols) x 2 feature halves (1024
rows) -- in the TRANSPOSED domain on-chip: per-core tensors are stored
[feature, batch] so the contraction dim of both matmuls lands on SBUF
partitions natively (w / wrec stay natural as the stationary operands,
column-halved per core).  The whole datapath is bfloat16 in HBM: the PE
runs bf16 at the same 1 column/cycle as fp32r, so quantizing everything
to bf16 halves DMA traffic at ~2.6e-3 relative error, far inside the
2e-2 gate.  PSUM accumulation and the threshold compare stay fp32.

The batch shard is exactly 512 columns, so each matmul's moving dim fills
one 2 KiB PSUM bank exactly: the two output quarters use disjoint banks
and the PE never waits on an epilogue.  Matmul phases run MM1q0, MM1q1,
MM2q0, MM2q1: the second phase reuses the resident x, so the z/z_out
streams for MM2's rhs (zon) have a whole extra phase to arrive -- this
makes the DMA-prefix deadline chain start-limited rather than
mid-stream-limited.  Weights stream on the scalar engine's HWDGE ring
while activations stream on sync's, so the two issue sequencers run in
parallel (each dma_start costs ~0.6 us of issue time) and the first
weight piece lands concurrently with the first x tiles.  Load
granularity follows the deadline slack: fine pieces where the PE is
supply-limited (start of MM1q0), 1 MiB pieces where there is multi-us
slack (wrec[1], other-half z streams) -- fewer dma_starts also shrink
the post-kernel notification flush, which scales with event count.
Stores go on the gpsimd SWDGE.  The final MM2q1 chunk runs n-major so
each PSUM bank completes early; the per-tile epilogue does the fp32
threshold compare first, then one fused op (v_out = ALPHA*pre + psum,
with pre = v - (V_TH/ALPHA)*z built mid-stream), so the mask store and
split v_out stores overlap the last matmuls.  Batch row 0 is column 0 of
the ms==0 shard, so those cores compute the real mask; z_new is rank-1
and only the [P, HT] mask leaves the device (host broadcasts it).  SPMD
uniformity across the feature halves is achieved purely in DATA: the
host permutes the tile order of z/z_out (own half first) and permutes
wrec's row blocks to match, so one program serves both halves.
"""

import sys

sys.path.insert(0, "/opt/trn_rl_repo")

import ml_dtypes
import numpy as np

import concourse.mybir as mybir
import concourse.tile as tile
from concourse import bacc, bass_utils

N = 2048
P = 128
NT = N // P          # 16 feature/contraction tiles
NCORES = 8
R, C = 4, 2          # batch shards x feature halves
MS = N // R          # 512-column batch shard == one PSUM bank of fp32
NH = N // C          # 1024-row feature half
HT = NH // P         # 8 n-tiles per half
KW = 8               # k-tiles per packed weight chunk (1 MiB chunks)
ALPHA = 1.0 - 0.05 / 10.0   # 0.995
BETA = 1.0 - 0.05 / 2.0     # 0.975
V_TH = 2.0

F32 = mybir.dt.float32
BF16 = mybir.dt.bfloat16
NP_BF16 = ml_dtypes.bfloat16


def _build_program():
    # bacc (not raw Bass): its compile pass splits multi-semaphore sync
    # waits that walrus's per-instruction wait limit rejects.
    nc = bacc.Bacc("TRN2", target_bir_lowering=False, debug=False, num_devices=NCORES)

    xt = nc.dram_tensor("xt", [P, NT, MS], BF16, kind="ExternalInput").ap()
    vt = nc.dram_tensor("vt", [P, HT, MS], BF16, kind="ExternalInput").ap()
    zt = nc.dram_tensor("zt", [P, NT, MS], BF16, kind="ExternalInput").ap()
    zot = nc.dram_tensor("zot", [P, NT, MS], BF16, kind="ExternalInput").ap()
    # chunk-major: [quarter, chunk, p, a, n] in exact DMA consumption order
    wh = nc.dram_tensor("wh", [2, NT // KW, P, KW, MS], BF16, kind="ExternalInput").ap()
    wrech = nc.dram_tensor(
        "wrech", [2, NT // KW, P, KW, MS], BF16, kind="ExternalInput"
    ).ap()

    vout = nc.dram_tensor("vout", [P, HT, MS], BF16, kind="ExternalOutput").ap()
    zoout = nc.dram_tensor("zoout", [P, HT, MS], BF16, kind="ExternalOutput").ap()
    maskout = nc.dram_tensor("maskout", [P, HT], F32, kind="ExternalOutput").ap()

    add = mybir.AluOpType.add
    mult = mybir.AluOpType.mult
    is_gt = mybir.AluOpType.is_gt

    with tile.TileContext(nc) as tc:
        with (
            tc.tile_pool(name="resident", bufs=1) as res,
            tc.tile_pool(name="zstream", bufs=2) as zs,
            tc.tile_pool(name="w2", bufs=2) as w2p,
            tc.tile_pool(name="w4", bufs=6) as w4p,
            tc.tile_pool(name="w8", bufs=2) as w8p,
            tc.tile_pool(name="psum", bufs=8, space="PSUM") as psum_pool,
            tc.tile_pool(name="epi", bufs=2) as epi,
        ):
            xt_s = res.tile([P, NT, MS], BF16, tag="xt_s")
            zt_s = res.tile([P, HT, MS], BF16, tag="zt_s")      # own half only
            vt_s = res.tile([P, HT, MS], BF16, tag="vt_s")
            zon = res.tile([P, NT, MS], BF16, tag="zon")        # matmul-2 rhs
            zot01 = res.tile([P, HT, MS], BF16, tag="zot01")    # own-half z_out
            pre = res.tile([P, HT, MS], F32, tag="pre")         # v - (V_TH/ALPHA)*z
            mask_s = res.tile([P, HT], F32, tag="mask_s")

            ps0 = [psum_pool.tile([P, MS], F32, tag="ps", name=f"ps0_{i}") for i in range(4)]
            ps1 = [psum_pool.tile([P, MS], F32, tag="ps", name=f"ps1_{i}") for i in range(4)]

            POOLS = {2: w2p, 4: w4p, 8: w8p}

            def wload(src, q, ci, a0, a1):
                """One weight piece [P, a1-a0, MS] on the scalar ring."""
                wc = POOLS[a1 - a0].tile([P, a1 - a0, MS], BF16, tag=f"wc{a1 - a0}")
                nc.scalar.dma_start(wc[:], src[q, ci, :, a0:a1, :])
                return wc

            def mm(wc, a, k, ps_n, rhs, start, stop):
                for n in range(4):
                    nc.tensor.matmul(
                        ps_n[n][:],
                        lhsT=wc[:, a, n * P : (n + 1) * P],
                        rhs=rhs[:, k, :],
                        start=start,
                        stop=stop,
                    )

            # --- MM1 q0: fine pieces, supply-limited phase.  Weight pieces
            # on the scalar ring land concurrently with x tiles on sync's.
            W0 = [(0, 0, 2), (0, 2, 4), (0, 4, 8), (1, 0, 4), (1, 4, 8)]
            XT = [(0, 2), (2, 4), (4, 8), (8, 12), (12, 16)]
            k = 0
            for i, (ci, a0, a1) in enumerate(W0):
                wc = wload(wh, 0, ci, a0, a1)
                t0, t1 = XT[i]
                nc.sync.dma_start(xt_s[:, t0:t1, :], xt[:, t0:t1, :])
                for a in range(a1 - a0):
                    mm(wc, a, k, ps0, xt_s, start=(k == 0), stop=False)
                    k += 1

            # --- MM1 q1: xt already resident, only wh[1] streams ---
            W1 = [(0, 0, 4), (0, 4, 8), (1, 0, 4), (1, 4, 8)]
            k = 0
            for ci, a0, a1 in W1:
                wc = wload(wh, 1, ci, a0, a1)
                for a in range(a1 - a0):
                    mm(wc, a, k, ps1, xt_s, start=(k == 0), stop=False)
                    k += 1

            # --- zon inputs (sync ring) + own-half builds ---
            nc.sync.dma_start(zt_s[:], zt[:, 0:HT, :])
            nc.sync.dma_start(zot01[:], zot[:, 0:HT, :])
            for j in range(HT):
                nc.vector.scalar_tensor_tensor(
                    zon[:, j, :], zot01[:, j, :], BETA, zt_s[:, j, :], mult, add
                )

            # --- MM2 q0: other-half z streams + wrec[0] pieces, interleaved
            # per 4-k-tile group so each group's semaphore fires early ---
            k = 0
            for i, (ci, a0, a1) in enumerate(W1):
                if i >= 2:
                    g0 = HT + 4 * (i - 2)
                    zt_q = zs.tile([P, 4, MS], BF16, tag="zt_q")
                    nc.sync.dma_start(zt_q[:], zt[:, g0 : g0 + 4, :])
                    zot_q = zs.tile([P, 4, MS], BF16, tag="zot_q")
                    nc.sync.dma_start(zot_q[:], zot[:, g0 : g0 + 4, :])
                    for a in range(4):
                        nc.vector.scalar_tensor_tensor(
                            zon[:, g0 + a, :], zot_q[:, a, :], BETA,
                            zt_q[:, a, :], mult, add,
                        )
                wc = wload(wrech, 0, ci, a0, a1)
                for a in range(a1 - a0):
                    mm(wc, a, k, ps0, zon, start=False, stop=(k == NT - 1))
                    k += 1

            nc.sync.dma_start(vt_s[:], vt[:])
            # pre = v - (V_TH/ALPHA)*z; epilogue applies *ALPHA so that
            # ALPHA*pre + psum = ALPHA*v - V_TH*z + matmuls
            for t in range(HT):
                nc.vector.scalar_tensor_tensor(
                    pre[:, t, :], zt_s[:, t, :], -V_TH / ALPHA, vt_s[:, t, :], mult, add
                )

            def epi_tile(t, psn, vob, nv):
                # fp32 threshold compare first, so the mask store need not
                # wait for the full-width add
                u = epi.tile([P, 1], F32, tag="u")
                nc.vector.scalar_tensor_tensor(
                    u[:], pre[:, t, 0:1], ALPHA, psn[:, 0:1], mult, add
                )
                nc.vector.tensor_scalar(mask_s[:, t : t + 1], u[:], V_TH, None, is_gt)
                nc.vector.scalar_tensor_tensor(
                    vob[:, nv, :], pre[:, t, :], ALPHA, psn[:], mult, add
                )

            vob0 = epi.tile([P, 4, MS], BF16, tag="vob", name="vob0")
            for n in range(4):
                epi_tile(n, ps0[n], vob0, n)
            nc.gpsimd.dma_start(zoout[:], zon[:, 0:HT, :])
            nc.gpsimd.dma_start(vout[:, 0:4, :], vob0[:])

            # --- MM2 q1: wrec[1] in 1 MiB pieces (multi-us deadline slack);
            # the final chunk runs n-major so each PSUM bank finishes early
            # and the epilogue/stores overlap the last matmuls ---
            wc = wload(wrech, 1, 0, 0, KW)
            for a in range(KW):
                mm(wc, a, a, ps1, zon, start=False, stop=False)
            wc = wload(wrech, 1, 1, 0, KW)
            vob1 = epi.tile([P, 4, MS], BF16, tag="vob", name="vob1")
            for n in range(4):
                for a in range(KW):
                    nc.tensor.matmul(
                        ps1[n][:],
                        lhsT=wc[:, a, n * P : (n + 1) * P],
                        rhs=zon[:, KW + a, :],
                        start=False,
                        stop=(a == KW - 1),
                    )
                epi_tile(4 + n, ps1[n], vob1, n)
                if n == 1:
                    nc.gpsimd.dma_start(vout[:, 4:6, :], vob1[:, 0:2, :])
            nc.gpsimd.dma_start(maskout[:], mask_s[:])
            nc.gpsimd.dma_start(vout[:, 6:8, :], vob1[:, 2:4, :])

    nc.compile()
    return nc


_PROGRAM_CACHE = {}


def _get_program():
    if "nc" not in _PROGRAM_CACHE:
        _PROGRAM_CACHE["nc"] = _build_program()
    return _PROGRAM_CACHE["nc"]


def _pack(aT, ms, tile_perm=None):
    """[2048, 2048] transposed-domain array -> p-major [128, T, MS] bf16."""
    a = aT[:, ms * MS : (ms + 1) * MS]  # [2048, MS]
    t = a.reshape(-1, P, MS)  # [T, 128, MS]
    if tile_perm is not None:
        t = t[tile_perm]
    return np.ascontiguousarray(t.transpose(1, 0, 2).astype(NP_BF16))


def _pack_w(w_h):
    """[2048, 1024] weight half -> chunk-major [2, 2, 128, KW, MS] bf16."""
    # w_h[ci*1024 + a*128 + p, q*512 + n] -> wp[q, ci, p, a, n]
    t = w_h.reshape(NT // KW, KW, P, 2, MS)
    return np.ascontiguousarray(t.transpose(3, 0, 2, 1, 4).astype(NP_BF16))


def make_in_maps(x, v, z, z_out, w, wrec):
    xT = np.ascontiguousarray(x.T)
    vT = np.ascontiguousarray(v.T)
    zT = np.ascontiguousarray(z.T)
    zoT = np.ascontiguousarray(z_out.T)
    w = np.ascontiguousarray(w, dtype=np.float32)
    wrec = np.ascontiguousarray(wrec, dtype=np.float32)

    wh_packed = [_pack_w(w[:, nh * NH : (nh + 1) * NH]) for nh in range(C)]
    wrech_packed = []
    for nh in range(C):
        perm = np.r_[nh * HT : nh * HT + HT, (1 - nh) * HT : (1 - nh) * HT + HT]
        wr = wrec.reshape(NT, P, N)[perm].reshape(N, N)[:, nh * NH : (nh + 1) * NH]
        wrech_packed.append(_pack_w(wr))

    in_maps = []
    for c in range(NCORES):
        nh, ms = divmod(c, R)
        perm = np.r_[nh * HT : nh * HT + HT, (1 - nh) * HT : (1 - nh) * HT + HT]
        in_maps.append(
            {
                "xt": _pack(xT, ms),
                "vt": _pack(vT, ms)[:, nh * HT : nh * HT + HT],
                "zt": _pack(zT, ms, perm),
                "zot": _pack(zoT, ms, perm),
                "wh": wh_packed[nh],
                "wrech": wrech_packed[nh],
            }
        )
    return in_maps


def gather(results):
    v_new = np.empty((N, N), np.float32)
    z_new = np.empty((N, N), np.float32)
    z_out_new = np.empty((N, N), np.float32)
    for c, r in enumerate(results):
        nh, ms = divmod(c, R)
        rows = slice(nh * NH, (nh + 1) * NH)
        cols = slice(ms * MS, (ms + 1) * MS)
        vo = np.asarray(r["vout"]).astype(np.float32).transpose(1, 0, 2).reshape(NH, MS)
        zo = np.asarray(r["zoout"]).astype(np.float32).transpose(1, 0, 2).reshape(NH, MS)
        v_new[cols, rows] = vo.T  # transposed domain -> natural
        z_out_new[cols, rows] = zo.T
        if ms == 0:
            # batch row 0 is column 0 of this shard: its mask is the real one.
            # mask feature t*128+p lives at maskout[p, t]; z_new is rank-1.
            mvec = np.asarray(r["maskout"]).astype(np.float32).T.reshape(NH)
            z_new[rows, :] = mvec[:, None]
    return v_new, z_new, z_out_new


def kernel(x, v, z, z_out, w, wrec, _trace=False):
    nc = _get_program()
    in_maps = make_in_maps(x, v, z, z_out, w, wrec)
    res = bass_utils.run_bass_kernel_spmd(
        nc, in_maps, core_ids=list(range(NCORES)), trace=_trace
    )
    out = gather(res.results)
    if _trace:
        return out, res
    return out


# revision 11
# speedup vs baseline: 1.0969x; 1.0926x over previous
"""Trainium2 Bass kernel for the LIF-network step (nn_NetworkClass_31018253812098).

Computation (reference, all fp32, N = NN = N_IN = 2048):
    z_out_new = BETA * z_out + z
    v_new     = ALPHA * v + x @ w - V_TH * z + z_out_new @ wrec
    mask      = (v_new[0, :] - V_TH) > 0          # length-2048, from batch row 0
    z_new[i, j] = mask[i]                         # row-broadcast (N == NN)

Strategy: 4x2 grid -- 4 batch shards (512 cols) x 2 feature halves (1024
rows) -- in the TRANSPOSED domain on-chip: per-core tensors are stored
[feature, batch] so the contraction dim of both matmuls lands on SBUF
partitions natively (w / wrec stay natural as the stationary operands,
column-halved per core).  The whole datapath is bfloat16 in HBM: the PE
runs bf16 at the same 1 column/cycle as fp32r, so quantizing everything
to bf16 halves DMA traffic at ~2.6e-3 relative error, far inside the
2e-2 gate.  PSUM accumulation and the threshold compare stay fp32.

The batch shard is exactly 512 columns, so each matmul's moving dim fills
one 2 KiB PSUM bank exactly: the two output quarters use disjoint banks
and the PE never waits on an epilogue.  Matmul phases run MM1q0, MM1q1,
MM2q0, MM2q1: the second phase reuses the resident x, so the z/z_out
streams for MM2's rhs (zon) have a whole extra phase to arrive -- this
makes the DMA-prefix deadline chain start-limited rather than
mid-stream-limited.  Weights stream on the scalar engine's HWDGE ring
while activations stream on sync's, so the two issue sequencers run in
parallel (each dma_start costs ~0.6 us of issue time) and the first
weight chunk lands concurrently with the first x tiles.  The final MM2q1
chunk runs n-major so each PSUM bank completes 4 matmuls before the
next, letting the one-op epilogue (v_out = ALPHA*pre + psum, with
pre = v - (V_TH/ALPHA)*z built mid-stream) and the split output stores
overlap the last matmuls.  Batch row 0 is column 0 of the ms==0 shard,
so those cores compute the real mask; z_new is rank-1 and only the
[P, HT] mask leaves the device (host broadcasts it).  SPMD uniformity
across the feature halves is achieved purely in DATA: the host permutes
the tile order of z/z_out (own half first) and permutes wrec's row
blocks to match, so one program serves both halves.
"""

import sys

sys.path.insert(0, "/opt/trn_rl_repo")

import ml_dtypes
import numpy as np

import concourse.mybir as mybir
import concourse.tile as tile
from concourse import bacc, bass_utils

N = 2048
P = 128
NT = N // P          # 16 feature/contraction tiles
NCORES = 8
R, C = 4, 2          # batch shards x feature halves
MS = N // R          # 512-column batch shard == one PSUM bank of fp32
NH = N // C          # 1024-row feature half
HT = NH // P         # 8 n-tiles per half
KC = 4               # k-tiles per weight chunk (512 KiB chunks)
ALPHA = 1.0 - 0.05 / 10.0   # 0.995
BETA = 1.0 - 0.05 / 2.0     # 0.975
V_TH = 2.0

F32 = mybir.dt.float32
BF16 = mybir.dt.bfloat16
NP_BF16 = ml_dtypes.bfloat16


def _build_program():
    # bacc (not raw Bass): its compile pass splits multi-semaphore sync
    # waits that walrus's per-instruction wait limit rejects.
    nc = bacc.Bacc("TRN2", target_bir_lowering=False, debug=False, num_devices=NCORES)

    xt = nc.dram_tensor("xt", [P, NT, MS], BF16, kind="ExternalInput").ap()
    vt = nc.dram_tensor("vt", [P, HT, MS], BF16, kind="ExternalInput").ap()
    zt = nc.dram_tensor("zt", [P, NT, MS], BF16, kind="ExternalInput").ap()
    zot = nc.dram_tensor("zot", [P, NT, MS], BF16, kind="ExternalInput").ap()
    # chunk-major: [quarter, kc, p, a, n] in exact DMA consumption order
    wh = nc.dram_tensor("wh", [2, NT // KC, P, KC, MS], BF16, kind="ExternalInput").ap()
    wrech = nc.dram_tensor(
        "wrech", [2, NT // KC, P, KC, MS], BF16, kind="ExternalInput"
    ).ap()

    vout = nc.dram_tensor("vout", [P, HT, MS], BF16, kind="ExternalOutput").ap()
    zoout = nc.dram_tensor("zoout", [P, HT, MS], BF16, kind="ExternalOutput").ap()
    maskout = nc.dram_tensor("maskout", [P, HT], F32, kind="ExternalOutput").ap()

    add = mybir.AluOpType.add
    mult = mybir.AluOpType.mult
    is_gt = mybir.AluOpType.is_gt

    with tile.TileContext(nc) as tc:
        with (
            tc.tile_pool(name="resident", bufs=1) as res,
            tc.tile_pool(name="zstream", bufs=2) as zs,
            tc.tile_pool(name="whalf", bufs=2) as whp,
            tc.tile_pool(name="wchunk", bufs=6) as wpool,
            tc.tile_pool(name="psum", bufs=8, space="PSUM") as psum_pool,
            tc.tile_pool(name="epi", bufs=2) as epi,
        ):
            xt_s = res.tile([P, NT, MS], BF16, tag="xt_s")
            zt_s = res.tile([P, HT, MS], BF16, tag="zt_s")      # own half only
            vt_s = res.tile([P, HT, MS], BF16, tag="vt_s")
            zon = res.tile([P, NT, MS], BF16, tag="zon")        # matmul-2 rhs
            zot01 = res.tile([P, HT, MS], BF16, tag="zot01")    # own-half z_out
            pre = res.tile([P, HT, MS], F32, tag="pre")         # v - (V_TH/ALPHA)*z
            mask_s = res.tile([P, HT], F32, tag="mask_s")

            ps0 = [psum_pool.tile([P, MS], F32, tag="ps", name=f"ps0_{i}") for i in range(4)]
            ps1 = [psum_pool.tile([P, MS], F32, tag="ps", name=f"ps1_{i}") for i in range(4)]

            # --- MM1 q0: first chunk split in half, weights on the scalar
            # ring so both HWDGE sequencers issue the critical path at once.
            wc0a = whp.tile([P, 2, MS], BF16, tag="wch")
            nc.scalar.dma_start(wc0a[:], wh[0, 0, :, 0:2, :])
            nc.sync.dma_start(xt_s[:, 0:2, :], xt[:, 0:2, :])
            wc0b = whp.tile([P, 2, MS], BF16, tag="wch")
            nc.scalar.dma_start(wc0b[:], wh[0, 0, :, 2:4, :])
            nc.sync.dma_start(xt_s[:, 2:4, :], xt[:, 2:4, :])
            for k in range(4):
                wc = wc0a if k < 2 else wc0b
                for n in range(4):
                    nc.tensor.matmul(
                        ps0[n][:],
                        lhsT=wc[:, k % 2, n * P : (n + 1) * P],
                        rhs=xt_s[:, k, :],
                        start=(k == 0),
                        stop=False,
                    )
            for kc in range(1, NT // KC):
                wc = wpool.tile([P, KC, MS], BF16, tag="wc")
                nc.scalar.dma_start(wc[:], wh[0, kc])
                nc.sync.dma_start(
                    xt_s[:, 4 * kc : 4 * kc + 4, :], xt[:, 4 * kc : 4 * kc + 4, :]
                )
                for a in range(KC):
                    k = kc * KC + a
                    for n in range(4):
                        nc.tensor.matmul(
                            ps0[n][:],
                            lhsT=wc[:, a, n * P : (n + 1) * P],
                            rhs=xt_s[:, k, :],
                            start=False,
                            stop=False,
                        )

            # --- MM1 q1: xt already resident, only wh[1] streams ---
            for kc in range(NT // KC):
                wc = wpool.tile([P, KC, MS], BF16, tag="wc")
                nc.scalar.dma_start(wc[:], wh[1, kc])
                for a in range(KC):
                    k = kc * KC + a
                    for n in range(4):
                        nc.tensor.matmul(
                            ps1[n][:],
                            lhsT=wc[:, a, n * P : (n + 1) * P],
                            rhs=xt_s[:, k, :],
                            start=(k == 0),
                            stop=False,
                        )

            # --- zon inputs (sync ring) + own-half builds ---
            nc.sync.dma_start(zt_s[:], zt[:, 0:HT, :])
            nc.sync.dma_start(zot01[:], zot[:, 0:HT, :])
            for j in range(HT):
                nc.vector.scalar_tensor_tensor(
                    zon[:, j, :], zot01[:, j, :], BETA, zt_s[:, j, :], mult, add
                )

            # --- MM2 q0: other-half z streams + wrec[0] chunks ---
            for kc in range(NT // KC):
                if kc >= 2:
                    zt_q = zs.tile([P, 4, MS], BF16, tag="zt_q")
                    nc.sync.dma_start(zt_q[:], zt[:, 4 * kc : 4 * kc + 4, :])
                    zot_q = zs.tile([P, 4, MS], BF16, tag="zot_q")
                    nc.sync.dma_start(zot_q[:], zot[:, 4 * kc : 4 * kc + 4, :])
                    for a in range(KC):
                        j = kc * KC + a
                        nc.vector.scalar_tensor_tensor(
                            zon[:, j, :], zot_q[:, a, :], BETA, zt_q[:, a, :], mult, add
                        )
                wc = wpool.tile([P, KC, MS], BF16, tag="wc")
                nc.scalar.dma_start(wc[:], wrech[0, kc])
                for a in range(KC):
                    k = kc * KC + a
                    for n in range(4):
                        nc.tensor.matmul(
                            ps0[n][:],
                            lhsT=wc[:, a, n * P : (n + 1) * P],
                            rhs=zon[:, k, :],
                            start=False,
                            stop=(k == NT - 1),
                        )

            # zoout = own-half zon, stored as soon as it is built
            nc.gpsimd.dma_start(zoout[:, 0:4, :], zon[:, 0:4, :])
            nc.gpsimd.dma_start(zoout[:, 4:8, :], zon[:, 4:8, :])

            nc.sync.dma_start(vt_s[:], vt[:])
            # pre = v - (V_TH/ALPHA)*z; epilogue applies *ALPHA so that
            # ALPHA*pre + psum = ALPHA*v - V_TH*z + matmuls
            for t in range(HT):
                nc.vector.scalar_tensor_tensor(
                    pre[:, t, :], zt_s[:, t, :], -V_TH / ALPHA, vt_s[:, t, :], mult, add
                )

            def epi_tile(t, psn, vob, nv):
                # fp32 threshold compare first, so the mask store need not
                # wait for the full-width add
                u = epi.tile([P, 1], F32, tag="u")
                nc.vector.scalar_tensor_tensor(
                    u[:], pre[:, t, 0:1], ALPHA, psn[:, 0:1], mult, add
                )
                nc.vector.tensor_scalar(mask_s[:, t : t + 1], u[:], V_TH, None, is_gt)
                nc.vector.scalar_tensor_tensor(
                    vob[:, nv, :], pre[:, t, :], ALPHA, psn[:], mult, add
                )

            vob0 = epi.tile([P, 4, MS], BF16, tag="vob", name="vob0")
            for n in range(4):
                epi_tile(n, ps0[n], vob0, n)
            nc.gpsimd.dma_start(vout[:, 0:4, :], vob0[:])

            # --- MM2 q1: wrec[1] chunks; final chunk n-major so each PSUM
            # bank finishes early and the epilogue/stores overlap the PE ---
            for kc in range(NT // KC - 1):
                wc = wpool.tile([P, KC, MS], BF16, tag="wc")
                nc.scalar.dma_start(wc[:], wrech[1, kc])
                for a in range(KC):
                    k = kc * KC + a
                    for n in range(4):
                        nc.tensor.matmul(
                            ps1[n][:],
                            lhsT=wc[:, a, n * P : (n + 1) * P],
                            rhs=zon[:, k, :],
                            start=False,
                            stop=False,
                        )
            wc = wpool.tile([P, KC, MS], BF16, tag="wc")
            nc.scalar.dma_start(wc[:], wrech[1, NT // KC - 1])
            vob1 = epi.tile([P, 4, MS], BF16, tag="vob", name="vob1")
            for n in range(4):
                for a in range(KC):
                    nc.tensor.matmul(
                        ps1[n][:],
                        lhsT=wc[:, a, n * P : (n + 1) * P],
                        rhs=zon[:, NT - KC + a, :],
                        start=False,
                        stop=(a == KC - 1),
                    )
                epi_tile(4 + n, ps1[n], vob1, n)
                if n == 1:
                    nc.gpsimd.dma_start(vout[:, 4:6, :], vob1[:, 0:2, :])
            nc.gpsimd.dma_start(maskout[:], mask_s[:])
            nc.gpsimd.dma_start(vout[:, 6:8, :], vob1[:, 2:4, :])

    nc.compile()
    return nc


_PROGRAM_CACHE = {}


def _get_program():
    if "nc" not in _PROGRAM_CACHE:
        _PROGRAM_CACHE["nc"] = _build_program()
    return _PROGRAM_CACHE["nc"]


def _pack(aT, ms, tile_perm=None):
    """[2048, 2048] transposed-domain array -> p-major [128, T, MS] bf16."""
    a = aT[:, ms * MS : (ms + 1) * MS]  # [2048, MS]
    t = a.reshape(-1, P, MS)  # [T, 128, MS]
    if tile_perm is not None:
        t = t[tile_perm]
    return np.ascontiguousarray(t.transpose(1, 0, 2).astype(NP_BF16))


def _pack_w(w_h):
    """[2048, 1024] weight half -> chunk-major [2, 4, 128, KC, MS] bf16."""
    # w_h[kc*512 + a*128 + p, q*512 + n] -> wp[q, kc, p, a, n]
    t = w_h.reshape(NT // KC, KC, P, 2, MS)
    return np.ascontiguousarray(t.transpose(3, 0, 2, 1, 4).astype(NP_BF16))


def make_in_maps(x, v, z, z_out, w, wrec):
    xT = np.ascontiguousarray(x.T)
    vT = np.ascontiguousarray(v.T)
    zT = np.ascontiguousarray(z.T)
    zoT = np.ascontiguousarray(z_out.T)
    w = np.ascontiguousarray(w, dtype=np.float32)
    wrec = np.ascontiguousarray(wrec, dtype=np.float32)

    wh_packed = [_pack_w(w[:, nh * NH : (nh + 1) * NH]) for nh in range(C)]
    wrech_packed = []
    for nh in range(C):
        perm = np.r_[nh * HT : nh * HT + HT, (1 - nh) * HT : (1 - nh) * HT + HT]
        wr = wrec.reshape(NT, P, N)[perm].reshape(N, N)[:, nh * NH : (nh + 1) * NH]
        wrech_packed.append(_pack_w(wr))

    in_maps = []
    for c in range(NCORES):
        nh, ms = divmod(c, R)
        perm = np.r_[nh * HT : nh * HT + HT, (1 - nh) * HT : (1 - nh) * HT + HT]
        in_maps.append(
            {
                "xt": _pack(xT, ms),
                "vt": _pack(vT, ms)[:, nh * HT : nh * HT + HT],
                "zt": _pack(zT, ms, perm),
                "zot": _pack(zoT, ms, perm),
                "wh": wh_packed[nh],
                "wrech": wrech_packed[nh],
            }
        )
    return in_maps


def gather(results):
    v_new = np.empty((N, N), np.float32)
    z_new = np.empty((N, N), np.float32)
    z_out_new = np.empty((N, N), np.float32)
    for c, r in enumerate(results):
        nh, ms = divmod(c, R)
        rows = slice(nh * NH, (nh + 1) * NH)
        cols = slice(ms * MS, (ms + 1) * MS)
        vo = np.asarray(r["vout"]).astype(np.float32).transpose(1, 0, 2).reshape(NH, MS)
        zo = np.asarray(r["zoout"]).astype(np.float32).transpose(1, 0, 2).reshape(NH, MS)
        v_new[cols, rows] = vo.T  # transposed domain -> natural
        z_out_new[cols, rows] = zo.T
        if ms == 0:
            # batch row 0 is column 0 of this shard: its mask is the real one.
            # mask feature t*128+p lives at maskout[p, t]; z_new is rank-1.
            mvec = np.asarray(r["maskout"]).astype(np.float32).T.reshape(NH)
            z_new[rows, :] = mvec[:, None]
    return v_new, z_new, z_out_new


def kernel(x, v, z, z_out, w, wrec, _trace=False):
    nc = _get_program()
    in_maps = make_in_maps(x, v, z, z_out, w, wrec)
    res = bass_utils.run_bass_kernel_spmd(
        nc, in_maps, core_ids=list(range(NCORES)), trace=_trace
    )
    out = gather(res.results)
    if _trace:
        return out, res
    return out
